# revision 1
# baseline (speedup 1.0000x reference)
"""Causal ALiBi sliding-window GQA attention block on 8 TRN2 NeuronCores.

Sharding: 2-way data parallel (batch) x 4-way tensor parallel (heads).
Core c handles batch b = c//4 and query heads [8*(c%4), 8*(c%4)+8)
(= kv heads [2*(c%4), 2*(c%4)+2)).  Each core computes its slice of the
QKV projections, windowed-causal ALiBi attention for its 8 heads, and a
partial output projection; the host sums the 4 TP partials per batch.

Kernel math layout (per core):
  - everything is computed transposed: xT [D,S] streams as the moving
    operand, qT/kT are built with head-dim on partitions so attention
    scores come out as sT[j,i] (j on partitions).
  - ALiBi bias is fused into the score matmul as 2 extra contraction
    rows: k-side aug rows [j; 1], q-side aug rows [slope/SCALE;
    -slope/SCALE*i - CSAFE/SCALE], so PSUM = qk + (bias+C)/SCALE and a
    single scale-only Exp activation produces the (unnormalized)
    softmax weights.  Per-column constants cancel in the softmax.
  - softmax denominator comes from a ones-column appended to v (PV
    matmul emits [o; sum] in one accumulation group).
  - masks are needed only on the block-diagonal (causal) and the
    window-edge block; everything else in the window is mask-free.
"""

import os
import sys
from contextlib import ExitStack

import numpy as np

import concourse.bass as bass
import concourse.bacc as bacc
import concourse.mybir as mybir
import concourse.tile as tile
from concourse.bass_utils import run_bass_kernel_spmd

F16 = mybir.dt.float16
BF16 = mybir.dt.bfloat16
F32 = mybir.dt.float32

# Problem shape (hardcoded; the harness always runs this config).
B, S, D = 2, 2048, 2048
H, HKV, DH = 32, 8, 64
WIN = 1024
SCALE = 1.0 / float(np.sqrt(DH))

N_CORES = 8
TP = 4                      # head-parallel ways
HLOC = H // TP              # 8 q heads per core
GLOC = HKV // TP            # 2 kv heads per core
EQ = HLOC * DH              # 512 q channels per core
EKV = GLOC * DH             # 128 kv channels per core
CSAFE = 0.0                 # exponent shift (cancels in softmax)


def _strip_taus(a, nstrip_t, wt):
    """j-tiles contributing to query strip a (4 i-tiles), with their
    valid column range inside the strip.  Returns list of
    (tau, c_lo, c_hi, is_diag, is_edge); a full-coverage tau is first so
    PSUM accumulation can start with a full 512-col write."""
    out = []
    for tau in range(max(0, 4 * a - wt), 4 * a + 4):
        t_lo = max(4 * a, tau)
        t_hi = min(4 * a + 3, tau + wt)
        if t_lo > t_hi or tau >= nstrip_t:
            continue
        c_lo = 128 * t_lo - 512 * a
        c_hi = 128 * (t_hi + 1) - 512 * a
        is_diag = 4 * a <= tau <= 4 * a + 3          # causal block at c_lo
        is_edge = (t_hi == tau + wt)                 # window-edge block at c_hi-128
        out.append((tau, c_lo, c_hi, is_diag, is_edge))
    full = [x for x in out if x[2] - x[1] == 512]
    assert full, f"strip {a} has no full-coverage tau"
    first = full[0]
    return [first] + [x for x in out if x is not first]


def build_program(s=S, d=D, win=WIN):
    """Emit the single-core SPMD program.  Returns (nc, names)."""
    nt = s // 128           # i/j tiles
    sc_n = s // 512         # 512-wide s chunks
    dc_n = d // 128         # contraction chunks for projections
    wt = win // 128
    nstrip = nt // 4

    nc = bacc.Bacc("TRN2", target_bir_lowering=False, debug=False,
                   num_devices=N_CORES)

    dram = {}

    def din(name, shape, dt):
        dram[name] = nc.dram_tensor(name, shape, dt, kind="ExternalInput").ap()
        return dram[name]

    xT = din("xT", [d, s], F16)
    wq = din("wq", [d, EQ], F16)
    wk = din("wk", [d, EKV], F16)
    wv = din("wv", [d, EKV], F16)
    wo = din("wo", [EQ, d], F16)
    qaug = din("qaug", [2 * HLOC, s], F16)
    kaug = din("kaug", [2, s], F16)
    biaspk = din("biaspk", [1, EQ + 2 * EKV], F16)
    ident = din("ident", [128, 128], F16)
    mlow32 = din("mlow32", [128, 128], F32)
    mlow16 = din("mlow16", [128, 128], F16)
    mhi16 = din("mhi16", [128, 128], F16)
    out_d = nc.dram_tensor("out", [s, d], F16, kind="ExternalOutput").ap()

    with tile.TileContext(nc) as tc, ExitStack() as ctx:
        P = ctx.enter_context
        consts = P(tc.tile_pool(name="consts", bufs=1))
        wpool = P(tc.tile_pool(name="wpool", bufs=1))
        xpool = P(tc.tile_pool(name="xpool", bufs=2))
        qapool = P(tc.tile_pool(name="qapool", bufs=1))
        vpool = P(tc.tile_pool(name="vpool", bufs=1))
        otpool = P(tc.tile_pool(name="otpool", bufs=1))
        work = P(tc.tile_pool(name="work", bufs=2))
        wexp = P(tc.tile_pool(name="wexp", bufs=3))
        nrm = P(tc.tile_pool(name="nrm", bufs=2))
        osbp = P(tc.tile_pool(name="osbp", bufs=3))
        psX = P(tc.tile_pool(name="psX", bufs=4, space="PSUM"))
        psPV = P(tc.tile_pool(name="psPV", bufs=1, space="PSUM"))

        # ---- weights (gpsimd SWDGE queue, parallel to sync-queue xt) ----
        wq_sb = wpool.tile([128, dc_n, EQ], F16, name="wq_sb")
        wq_r = wq.rearrange("(c p) e -> p c e", p=128)
        for dq in range(4):
            q4w = dc_n // 4
            nc.gpsimd.dma_start(wq_sb[:, dq * q4w:(dq + 1) * q4w, :],
                                wq_r[:, dq * q4w:(dq + 1) * q4w, :])
        wk_sb = wpool.tile([128, dc_n, EKV], F16, name="wk_sb")
        nc.gpsimd.dma_start(wk_sb[:], wk.rearrange("(c p) e -> p c e", p=128))
        wv_sb = wpool.tile([128, dc_n, EKV], F16, name="wv_sb")
        nc.gpsimd.dma_start(wv_sb[:], wv.rearrange("(c p) e -> p c e", p=128))
        bias_sb = consts.tile([1, EQ + 2 * EKV], F16, name="bias_sb")
        nc.gpsimd.dma_start(bias_sb[:], biaspk[:])
        ones_row = consts.tile([1, 512], F16, name="ones_row")
        nc.vector.memset(ones_row[:], 1.0)
        ones_f32 = consts.tile([1, 512], F32, name="ones_f32")
        nc.vector.memset(ones_f32[:], 1.0)
        ones_col = consts.tile([1, 128], F16, name="ones_col")
        nc.vector.memset(ones_col[:], 1.0)
        ident_sb = consts.tile([128, 128], F16, name="ident_sb")
        nc.gpsimd.dma_start(ident_sb[:], ident[:])
        ml32_sb = consts.tile([128, 128], F32, name="ml32_sb")
        nc.gpsimd.dma_start(ml32_sb[:], mlow32[:])
        ml16_sb = consts.tile([128, 128], F16, name="ml16_sb")
        nc.gpsimd.dma_start(ml16_sb[:], mlow16[:])
        mh16_sb = consts.tile([128, 128], F16, name="mh16_sb")
        nc.gpsimd.dma_start(mh16_sb[:], mhi16[:])
        # wo is first needed by the deferred output projection (after
        # attention strip 1) -- load it late on the gpsimd queue.
        wo_sb = wpool.tile([128, EQ // 128, d], F16, name="wo_sb")
        nc.gpsimd.dma_start(wo_sb[:], wo.rearrange("(c p) e -> p c e", p=128))

        # ---- persistent activation tensors ----
        qa = []
        for h in range(HLOC):
            t = qapool.tile([128, s], F16, name=f"qa{h}")
            nc.vector.memset(t[64:128, :], 0.0)
            nc.sync.dma_start(t[64:66, :], qaug[2 * h:2 * h + 2, :])
            qa.append(t)
        ka = []
        for g in range(GLOC):
            t = qapool.tile([128, s], F16, name=f"ka{g}")
            nc.vector.memset(t[64:128, :], 0.0)
            nc.sync.dma_start(t[64:66, :], kaug[:, :])
            ka.append(t)
        va = []
        for g in range(GLOC):
            t = vpool.tile([128, nt, 128], F16, name=f"va{g}")
            nc.vector.memset(t[:, :, 64:128], 0.0)
            nc.vector.memset(t[:, :, 64:65], 1.0)
            va.append(t)
        oT = []
        for ec in range(EQ // 128):
            t = otpool.tile([128, s], F16, name=f"oT{ec}")
            oT.append(t)

        # ---------- phase 1 emitter: projections for one s-chunk ----------
        def emit_proj_chunk(sc):
            xt = xpool.tile([128, dc_n, 512], F16, name="xt", tag="xt")
            q4 = dc_n // 4
            for dq in range(4):
                nc.sync.dma_start(
                    xt[:, dq * q4:(dq + 1) * q4, :],
                    xT[dq * q4 * 128:(dq + 1) * q4 * 128,
                       sc * 512:(sc + 1) * 512]
                    .rearrange("(c p) s -> p c s", p=128))
            for et in range(EQ // 128 + 2):
                ps = psX.tile([128, 512], F32, name="ps_proj", tag="mm")
                if et < EQ // 128:
                    w_lhs = lambda dc: wq_sb[:, dc, et * 128:(et + 1) * 128]
                    b_lhs = bias_sb[0:1, et * 128:(et + 1) * 128]
                elif et == EQ // 128:
                    w_lhs = lambda dc: wk_sb[:, dc, :]
                    b_lhs = bias_sb[0:1, EQ:EQ + EKV]
                else:
                    w_lhs = lambda dc: wv_sb[:, dc, :]
                    b_lhs = bias_sb[0:1, EQ + EKV:EQ + 2 * EKV]
                for dc in range(dc_n):
                    nc.tensor.matmul(ps[:], w_lhs(dc), xt[:, dc, :],
                                     start=(dc == 0), stop=False)
                nc.tensor.matmul(ps[:], b_lhs, ones_row[:],
                                 start=False, stop=True)
                cols = slice(sc * 512, (sc + 1) * 512)
                if et < EQ // 128:
                    nc.vector.tensor_copy(qa[2 * et][0:64, cols], ps[0:64, :])
                    nc.vector.tensor_copy(qa[2 * et + 1][0:64, cols], ps[64:128, :])
                elif et == EQ // 128:
                    nc.vector.tensor_copy(ka[0][0:64, cols], ps[0:64, :])
                    nc.vector.tensor_copy(ka[1][0:64, cols], ps[64:128, :])
                else:
                    vt = work.tile([128, 512], F16, name="vt", tag="vt")
                    nc.vector.tensor_copy(vt[:], ps[:])
                    for jt in range(4):
                        pst = psX.tile([128, 128], F16, name="ps_tr", tag="mm")
                        nc.tensor.transpose(pst[:], vt[:, jt * 128:(jt + 1) * 128],
                                            ident_sb[:])
                        jg = sc * 4 + jt
                        nc.vector.tensor_copy(va[0][:, jg, 0:64], pst[:, 0:64])
                        nc.vector.tensor_copy(va[1][:, jg, 0:64], pst[:, 64:128])

        # ---------- phase 2 emitters ----------
        def emit_normalize(a, g, hp, pvs):
            # o[dh,i] = pv[dh,i] / pv[64,i]
            for u in range(2):
                h = g * 4 + hp * 2 + u
                dn = nrm.tile([1, 512], F32, name="dn", tag="dn")
                nc.vector.tensor_copy(dn[:], pvs[u][64:65, :])
                rc = nrm.tile([1, 512], F32, name="rc", tag="rc")
                nc.vector.reciprocal(rc[:], dn[:])
                rc16 = nrm.tile([1, 512], F16, name="rc16", tag="rc16")
                nc.scalar.copy(rc16[:], rc[:])
                # broadcast recip across 64 partitions: rank-1 matmul
                rbp = psX.tile([128, 512], F32, name="rbp", tag="mm")
                nc.tensor.matmul(rbp[:], ones_col[:], rc16[:],
                                 start=True, stop=True)
                rcb = nrm.tile([64, 512], F32, name="rcb", tag="rcb")
                nc.scalar.copy(rcb[:], rbp[0:64, :])
                r0 = (h % 2) * 64
                nc.vector.tensor_mul(
                    oT[h // 2][r0:r0 + 64, a * 512:(a + 1) * 512],
                    pvs[u][0:64, :], rcb[:])

        norm_pending = []   # deferred (a, g, hp, pvs)

        def flush_norms(keep=0):
            while len(norm_pending) > keep:
                emit_normalize(*norm_pending.pop(0))

        def emit_attn_pair(a, g, hp, taus):
            pvs = []
            for u in range(2):
                pv = psPV.tile([128, 512], F32, name=f"pv{u}",
                               tag=f"pv{u}", bufs=2)
                pvs.append(pv)
            # software pipeline: PV runs two taus behind the scores so the
            # PE never waits on the Exp.
            pend = []        # [(tau, c_lo, c_hi, [w_u0, w_u1], n), ...]
            first = True
            for (tau, c_lo, c_hi, is_diag, is_edge) in taus:
                n = c_hi - c_lo
                wts = []
                for u in range(2):
                    h = g * 4 + hp * 2 + u
                    pss = psX.tile([128, 512], F32, name="ps_s", tag="mm")
                    nc.tensor.matmul(
                        pss[:, 0:n],
                        ka[g][:, tau * 128:(tau + 1) * 128],
                        qa[h][:, 512 * a + c_lo:512 * a + c_hi],
                        start=True, stop=True)
                    if is_diag:
                        nc.vector.tensor_mul(pss[:, 0:128], pss[:, 0:128],
                                             ml32_sb[:])
                    w_t = wexp.tile([128, 512], F16, name=f"w{u}",
                                    tag=f"w{u}")
                    nc.scalar.activation(
                        w_t[:, 0:n], pss[:, 0:n],
                        mybir.ActivationFunctionType.Exp, scale=SCALE)
                    if is_diag:
                        nc.vector.tensor_mul(w_t[:, 0:128], w_t[:, 0:128],
                                             ml16_sb[:])
                    if is_edge:
                        nc.vector.tensor_mul(w_t[:, n - 128:n],
                                             w_t[:, n - 128:n], mh16_sb[:])
                    wts.append(w_t)
                if len(pend) >= 2:
                    ptau, pc_lo, pc_hi, pw, pn = pend.pop(0)
                    for u in range(2):
                        nc.tensor.matmul(
                            pvs[u][:, pc_lo:pc_hi],
                            va[g][:, ptau, :], pw[u][:, 0:pn],
                            start=(ptau == taus[0][0]), stop=False)
                if first:
                    # older pairs' normalizes hide under this pair's work
                    flush_norms(keep=1)
                    first = False
                pend.append((tau, c_lo, c_hi, wts, n))
            while pend:
                ptau, pc_lo, pc_hi, pw, pn = pend.pop(0)
                for u in range(2):
                    nc.tensor.matmul(pvs[u][:, pc_lo:pc_hi],
                                     va[g][:, ptau, :], pw[u][:, 0:pn],
                                     start=(ptau == taus[0][0]),
                                     stop=(not pend))
            norm_pending.append((a, g, hp, pvs))

        def emit_attn_strip(a):
            taus = _strip_taus(a, nt, wt)
            for g in range(GLOC):
                for hp in range(2):
                    emit_attn_pair(a, g, hp, taus)

        def emit_oproj_strip(a):
            for st in range(4 * a, 4 * a + 4):
                for dcb in range(d // 512):
                    ps = psX.tile([128, 512], F32, name="ps_o", tag="mm")
                    for ec in range(EQ // 128):
                        nc.tensor.matmul(
                            ps[:], oT[ec][:, st * 128:(st + 1) * 128],
                            wo_sb[:, ec, dcb * 512:(dcb + 1) * 512],
                            start=(ec == 0), stop=(ec == EQ // 128 - 1))
                    osb = osbp.tile([128, 512], F16, name="osb", tag="osb")
                    nc.scalar.copy(osb[:], ps[:])
                    nc.sync.dma_start(
                        out_d[st * 128:(st + 1) * 128,
                              dcb * 512:(dcb + 1) * 512], osb[:])

        # ---------- schedule ----------
        for sc in range(sc_n):
            emit_proj_chunk(sc)
        for a in range(nstrip):
            emit_attn_strip(a)
            if a > 0:
                emit_oproj_strip(a - 1)
        flush_norms()
        emit_oproj_strip(nstrip - 1)

    nc.compile()
    return nc


# ---------------- host-side sharding ----------------

def _prep_core_inputs(c, x, Wq, bq, Wk, bk, Wv, bv, Wo, slopes, s=S, d=D):
    """Build the per-core input map (all numpy, fp16 where declared)."""
    b = c // TP
    hs = c % TP
    f16 = np.float16
    qrows = slice(hs * EQ, (hs + 1) * EQ)
    krows = slice(hs * EKV, (hs + 1) * EKV)
    m = {}
    m["xT"] = np.ascontiguousarray(x[b].T).astype(f16)
    m["wq"] = np.ascontiguousarray(Wq[qrows, :].T).astype(f16)
    m["wk"] = np.ascontiguousarray(Wk[krows, :].T).astype(f16)
    m["wv"] = np.ascontiguousarray(Wv[krows, :].T).astype(f16)
    m["wo"] = np.ascontiguousarray(Wo[:, qrows].T).astype(f16)
    qaug = np.zeros((2 * HLOC, s), np.float32)
    i_idx = np.arange(s, dtype=np.float32)
    for h in range(HLOC):
        sl = float(slopes[hs * HLOC + h])
        qaug[2 * h, :] = sl / SCALE
        qaug[2 * h + 1, :] = -sl / SCALE * i_idx - CSAFE / SCALE
    m["qaug"] = qaug.astype(f16)
    kaug = np.zeros((2, s), np.float32)
    kaug[0, :] = i_idx
    kaug[1, :] = 1.0
    m["kaug"] = kaug.astype(f16)
    bpk = np.concatenate([bq[qrows], bk[krows], bv[krows]]).astype(f16)
    m["biaspk"] = bpk.reshape(1, -1)
    m["ident"] = np.eye(128, dtype=f16)
    p = np.arange(128)[:, None]
    f = np.arange(128)[None, :]
    m["mlow32"] = (p <= f).astype(np.float32)
    m["mlow16"] = (p <= f).astype(f16)
    m["mhi16"] = (p > f).astype(f16)
    return m


_PROG_CACHE = {}


def _get_program():
    key = (S, D, WIN)
    if key not in _PROG_CACHE:
        _PROG_CACHE[key] = build_program()
    return _PROG_CACHE[key]


def kernel(hidden_states, Wq, bq, Wk, bk, Wv, bv, Wo, bo, alibi_slopes,
           _want_profile=False):
    x = np.asarray(hidden_states, np.float32)
    Wq = np.asarray(Wq, np.float32)
    Wk = np.asarray(Wk, np.float32)
    Wv = np.asarray(Wv, np.float32)
    Wo = np.asarray(Wo, np.float32)
    bq = np.asarray(bq, np.float32)
    bk = np.asarray(bk, np.float32)
    bv = np.asarray(bv, np.float32)
    bo = np.asarray(bo, np.float32)
    slopes = np.asarray(alibi_slopes, np.float32)

    nc = _get_program()
    in_maps = [
        _prep_core_inputs(c, x, Wq, bq, Wk, bk, Wv, bv, Wo, slopes)
        for c in range(N_CORES)
    ]
    res = run_bass_kernel_spmd(nc, in_maps, list(range(N_CORES)),
                               trace=_want_profile)
    out = np.zeros((B, S, D), np.float32)
    for c in range(N_CORES):
        out[c // TP] += res.results[c]["out"].astype(np.float32)
    out += bo[None, None, :]
    if _want_profile:
        return out, res
    return out



# revision 7
# speedup vs baseline: 1.6585x; 1.6585x over previous
"""Causal ALiBi sliding-window GQA attention block on 8 TRN2 NeuronCores.

Sharding: 2-way data parallel (batch) x 4-way tensor parallel (heads).
Core c handles batch b = c//4 and query heads [8*(c%4), 8*(c%4)+8)
(= kv heads [2*(c%4), 2*(c%4)+2)).  Each core computes its slice of the
QKV projections, windowed-causal ALiBi attention for its 8 heads, and a
partial output projection; the host sums the 4 TP partials per batch.

Kernel math layout (per core):
  - everything is computed transposed: xT [D,S] streams as the moving
    operand, qT/kT are built with head-dim on partitions so attention
    scores come out as sT[j,i] (j on partitions).
  - ALiBi bias is fused into the score matmul as 2 extra contraction
    rows: k-side aug rows [j; 1], q-side aug rows [slope/SCALE;
    -slope/SCALE*i], so PSUM = qk + bias/SCALE and a single scale-only
    Exp activation produces the (unnormalized) softmax weights.
  - causal-diagonal and window-edge masks are applied by the PE itself:
    an extra matmul (identity stationary x const -30000 tile) adds a
    large negative into the masked positions of the score PSUM, so the
    Exp flushes them to exactly 0.  No vector-engine masking.
  - softmax denominator comes from a ones-column appended to v (PV
    matmul emits [o; sum] in one accumulation group); the reciprocal is
    a single fast-approx DVE op, broadcast across partitions by the
    (otherwise idle) gpsimd engine.
  - v is projected directly into [j, dh] layout (x.T @ Wv with the
    x-tile stationary), so no PE transposes are needed.
"""

import numpy as np

from contextlib import ExitStack

import concourse.bass as bass
import concourse.bacc as bacc
import concourse.mybir as mybir
import concourse.tile as tile
from concourse.bass_utils import run_bass_kernel_spmd

F16 = mybir.dt.float16
F32 = mybir.dt.float32

# Problem shape (hardcoded; the harness always runs this config).
B, S, D = 2, 2048, 2048
H, HKV, DH = 32, 8, 64
WIN = 1024
SCALE = 1.0 / float(np.sqrt(DH))
MNEG = -30000.0             # pre-exp additive mask (exp -> 0 in f16)

N_CORES = 8
TP = 4                      # head-parallel ways
HLOC = H // TP              # 8 q heads per core
GLOC = HKV // TP            # 2 kv heads per core
EQ = HLOC * DH              # 512 q channels per core
EKV = GLOC * DH             # 128 kv channels per core


def _strip_taus(a, nstrip_t, wt):
    """j-tiles contributing to query strip a (4 i-tiles), with their
    valid column range inside the strip.  Returns list of
    (tau, c_lo, c_hi, is_diag, is_edge); a full-coverage tau is first so
    PSUM accumulation can start with a full 512-col write."""
    out = []
    for tau in range(max(0, 4 * a - wt), 4 * a + 4):
        t_lo = max(4 * a, tau)
        t_hi = min(4 * a + 3, tau + wt)
        if t_lo > t_hi or tau >= nstrip_t:
            continue
        c_lo = 128 * t_lo - 512 * a
        c_hi = 128 * (t_hi + 1) - 512 * a
        is_diag = 4 * a <= tau <= 4 * a + 3          # causal block at c_lo
        is_edge = (t_hi == tau + wt)                 # window-edge block at c_hi-128
        out.append((tau, c_lo, c_hi, is_diag, is_edge))
    full = [x for x in out if x[2] - x[1] == 512]
    assert full, f"strip {a} has no full-coverage tau"
    first = full[0]
    return [first] + [x for x in out if x is not first]


def _strip_chunks(a, nstrip_t, wt):
    """Group the strip's taus into score-PSUM chunks of <= 512 columns;
    each chunk is one PSUM tile / one Exp activation.  Tau column ranges
    overlap in strip coordinates, so each tau gets a flattened offset
    `off` inside its chunk tile: entries are
    (tau, c_lo, c_hi, is_diag, is_edge, off)."""
    taus = _strip_taus(a, nstrip_t, wt)
    chunks, cur, cw = [], [], 0
    for t in taus:
        w = t[2] - t[1]
        if cw + w > 512:
            chunks.append(cur)
            cur, cw = [], 0
        cur.append(t + (cw,))
        cw += w
    if cur:
        chunks.append(cur)
    return chunks


def build_program(s=S, d=D, win=WIN):
    """Emit the single-core SPMD program.  Returns nc."""
    nt = s // 128           # i/j tiles
    dc_n = d // 128         # contraction chunks for projections
    wt = win // 128
    nstrip = nt // 4

    nc = bacc.Bacc("TRN2", target_bir_lowering=False, debug=False,
                   num_devices=N_CORES)

    dram = {}

    def din(name, shape, dt):
        dram[name] = nc.dram_tensor(name, shape, dt, kind="ExternalInput").ap()
        return dram[name]

    xT = din("xT", [d, s], F16)
    wq = din("wq", [d, EQ], F16)
    wk = din("wk", [d, EKV], F16)
    wv = din("wv", [d, EKV], F16)
    wo = din("wo", [EQ, d], F16)
    qaug = din("qaug", [2 * HLOC, s], F16)
    kaug = din("kaug", [2, s], F16)
    biascol = din("biascol", [128, 5], F32)   # q0..q3, k per-partition bias
    vbias = din("vbias", [1, EKV], F16)
    ident = din("ident", [128, 128], F16)
    mnlo = din("mnlo", [128, 128], F16)       # diag: 0 if p<=f else MNEG
    mnhi = din("mnhi", [128, 128], F16)       # edge: 0 if p>f else MNEG
    out_d = nc.dram_tensor("out", [s, d], F16, kind="ExternalOutput").ap()

    with tile.TileContext(nc) as tc, ExitStack() as ctx:
        P = ctx.enter_context
        consts = P(tc.tile_pool(name="consts", bufs=1))
        wpool = P(tc.tile_pool(name="wpool", bufs=1))
        xpool = P(tc.tile_pool(name="xpool", bufs=2))
        qapool = P(tc.tile_pool(name="qapool", bufs=1))
        vpool = P(tc.tile_pool(name="vpool", bufs=1))
        otpool = P(tc.tile_pool(name="otpool", bufs=1))
        wexp = P(tc.tile_pool(name="wexp", bufs=3))
        nrm = P(tc.tile_pool(name="nrm", bufs=2))
        osbp = P(tc.tile_pool(name="osbp", bufs=3))
        psX = P(tc.tile_pool(name="psX", bufs=4, space="PSUM"))
        psPV = P(tc.tile_pool(name="psPV", bufs=1, space="PSUM"))

        # ---- weights: wq split in quarters on the gpsimd SWDGE queue ----
        wq_sb = wpool.tile([128, dc_n, EQ], F16, name="wq_sb")
        wq_r = wq.rearrange("(c p) e -> p c e", p=128)
        for dq in range(4):
            q4w = dc_n // 4
            nc.gpsimd.dma_start(wq_sb[:, dq * q4w:(dq + 1) * q4w, :],
                                wq_r[:, dq * q4w:(dq + 1) * q4w, :])
        wk_sb = wpool.tile([128, dc_n, EKV], F16, name="wk_sb")
        nc.gpsimd.dma_start(wk_sb[:], wk.rearrange("(c p) e -> p c e", p=128))
        wv_sb = wpool.tile([128, dc_n, EKV], F16, name="wv_sb")
        nc.gpsimd.dma_start(wv_sb[:], wv.rearrange("(c p) e -> p c e", p=128))
        # small consts on the scalar queue (idle at startup)
        bias_sb = consts.tile([128, 5], F32, name="bias_sb")
        nc.scalar.dma_start(bias_sb[:], biascol[:])
        vbias_sb = consts.tile([1, EKV], F16, name="vbias_sb")
        nc.scalar.dma_start(vbias_sb[:], vbias[:])
        ident_sb = consts.tile([128, 128], F16, name="ident_sb")
        nc.scalar.dma_start(ident_sb[:], ident[:])
        mnlo_sb = consts.tile([128, 128], F16, name="mnlo_sb")
        nc.scalar.dma_start(mnlo_sb[:], mnlo[:])
        mnhi_sb = consts.tile([128, 128], F16, name="mnhi_sb")
        nc.scalar.dma_start(mnhi_sb[:], mnhi[:])
        ones_col = consts.tile([1, 128], F16, name="ones_col")
        nc.vector.memset(ones_col[:], 1.0)
        # wo is first needed by the deferred output projection (after
        # attention strip 1) -- load it late on the gpsimd queue.
        wo_sb = wpool.tile([128, EQ // 128, d], F16, name="wo_sb")
        nc.gpsimd.dma_start(wo_sb[:], wo.rearrange("(c p) e -> p c e", p=128))

        # ---- persistent activation tensors ----
        qa = []
        for h in range(HLOC):
            t = qapool.tile([128, s], F16, name=f"qa{h}")
            nc.vector.memset(t[64:128, :], 0.0)
            nc.scalar.dma_start(t[64:66, :], qaug[2 * h:2 * h + 2, :])
            qa.append(t)
        ka = []
        for g in range(GLOC):
            t = qapool.tile([128, s], F16, name=f"ka{g}")
            nc.vector.memset(t[64:128, :], 0.0)
            nc.scalar.dma_start(t[64:66, :], kaug[:, :])
            ka.append(t)
        va = []
        for g in range(GLOC):
            t = vpool.tile([128, nt, 65], F16, name=f"va{g}")
            nc.vector.memset(t[:, :, 64:65], 1.0)
            va.append(t)
        oT = []
        for ec in range(EQ // 128):
            t = otpool.tile([128, s], F16, name=f"oT{ec}")
            oT.append(t)

        # ---------- phase 1 emitter: projections for one s-chunk ----------
        def emit_proj_chunk(sc):
            xt = xpool.tile([128, dc_n, 512], F16, name="xt", tag="xt")
            q4 = dc_n // 4
            for dq in range(4):
                nc.sync.dma_start(
                    xt[:, dq * q4:(dq + 1) * q4, :],
                    xT[dq * q4 * 128:(dq + 1) * q4 * 128,
                       sc * 512:(sc + 1) * 512]
                    .rearrange("(c p) s -> p c s", p=128))
            cols = slice(sc * 512, (sc + 1) * 512)
            # q (4 x 128 chans) and k (128 chans): [chan, s] layout
            for et in range(EQ // 128 + 1):
                ps = psX.tile([128, 512], F32, name="ps_proj", tag="mm")
                if et < EQ // 128:
                    w_lhs = lambda dc: wq_sb[:, dc, et * 128:(et + 1) * 128]
                else:
                    w_lhs = lambda dc: wk_sb[:, dc, :]
                for dc in range(dc_n):
                    nc.tensor.matmul(ps[:], w_lhs(dc), xt[:, dc, :],
                                     start=(dc == 0), stop=(dc == dc_n - 1))
                if et < EQ // 128:
                    nc.vector.tensor_scalar_add(
                        qa[2 * et][0:64, cols], ps[0:64, :],
                        bias_sb[0:64, et:et + 1])
                    nc.vector.tensor_scalar_add(
                        qa[2 * et + 1][0:64, cols], ps[64:128, :],
                        bias_sb[64:128, et:et + 1])
                else:
                    nc.vector.tensor_scalar_add(
                        ka[0][0:64, cols], ps[0:64, :], bias_sb[0:64, 4:5])
                    nc.vector.tensor_scalar_add(
                        ka[1][0:64, cols], ps[64:128, :], bias_sb[64:128, 4:5])
            # v directly in [j, chan] layout: x-tile stationary
            for jt in range(4):
                jg = sc * 4 + jt
                psv = psX.tile([128, 512], F32, name="ps_v", tag="mm")
                for dc in range(dc_n):
                    nc.tensor.matmul(
                        psv[:, 0:EKV],
                        xt[:, dc, jt * 128:(jt + 1) * 128],
                        wv_sb[:, dc, :],
                        start=(dc == 0), stop=False)
                nc.tensor.matmul(psv[:, 0:EKV], ones_col[:], vbias_sb[:],
                                 start=False, stop=True)
                nc.vector.tensor_copy(va[0][:, jg, 0:64], psv[:, 0:64])
                nc.vector.tensor_copy(va[1][:, jg, 0:64], psv[:, 64:128])

        # ---------- phase 2 emitters ----------
        def emit_normalize(a, g, hp, pvs):
            # o[dh,i] = pv[dh,i] / pv[64,i]
            for u in range(2):
                h = g * 4 + hp * 2 + u
                dn = nrm.tile([1, 512], F32, name="dn", tag="dn")
                nc.vector.tensor_copy(dn[:], pvs[u][64:65, :])
                rc = nrm.tile([1, 512], F32, name="rc", tag="rc")
                # custom-DVE ops read SBUF only -- dn must not be PSUM
                nc.vector.reciprocal_approx_fast(rc[:], dn[:])
                rcb = nrm.tile([64, 512], F32, name="rcb", tag="rcb")
                nc.gpsimd.partition_broadcast(rcb[:], rc[:], channels=64)
                r0 = (h % 2) * 64
                nc.vector.tensor_mul(
                    oT[h // 2][r0:r0 + 64, a * 512:(a + 1) * 512],
                    pvs[u][0:64, :], rcb[:])

        norm_pending = []   # deferred (a, g, hp, pvs)

        def flush_norms(keep=0):
            while len(norm_pending) > keep:
                emit_normalize(*norm_pending.pop(0))

        def emit_attn_pair(a, g, hp, chunks):
            pvs = []
            for u in range(2):
                pv = psPV.tile([128, 512], F32, name=f"pv{u}",
                               tag=f"pv{u}", bufs=2)
                pvs.append(pv)
            first_tau = chunks[0][0][0]
            last_tau = chunks[-1][-1][0]

            def drain_one(pend):
                ctaus, wts = pend.pop(0)
                for (tau, c_lo, c_hi, _d, _e, off) in ctaus:
                    for u in range(2):
                        nc.tensor.matmul(
                            pvs[u][0:65, c_lo:c_hi],
                            va[g][:, tau, :],
                            wts[u][:, off:off + c_hi - c_lo],
                            start=(tau == first_tau),
                            stop=(tau == last_tau and u == 1))

            # software pipeline: PV runs two chunks behind the scores so
            # the PE never waits on the Exp.
            pend = []        # [(chunk_taus, [w_u0, w_u1]), ...]
            first = True
            for ctaus in chunks:
                cw = sum(t[2] - t[1] for t in ctaus)
                wts = []
                for u in range(2):
                    h = g * 4 + hp * 2 + u
                    ps = psX.tile([128, 512], F32, name="ps_s", tag="mm")
                    for (tau, c_lo, c_hi, is_diag, is_edge, off) in ctaus:
                        ka_t = ka[g][:, tau * 128:(tau + 1) * 128]
                        lo, hi = off, off + c_hi - c_lo
                        qs = 512 * a + c_lo
                        if is_diag:
                            nc.tensor.matmul(
                                ps[:, lo:lo + 128], ka_t,
                                qa[h][:, qs:qs + 128],
                                start=True, stop=False)
                            nc.tensor.matmul(
                                ps[:, lo:lo + 128], ident_sb[:], mnlo_sb[:],
                                start=False, stop=True)
                            if hi > lo + 128:
                                nc.tensor.matmul(
                                    ps[:, lo + 128:hi], ka_t,
                                    qa[h][:, qs + 128:512 * a + c_hi],
                                    start=True, stop=True)
                        elif is_edge:
                            if hi - 128 > lo:
                                nc.tensor.matmul(
                                    ps[:, lo:hi - 128], ka_t,
                                    qa[h][:, qs:512 * a + c_hi - 128],
                                    start=True, stop=True)
                            nc.tensor.matmul(
                                ps[:, hi - 128:hi], ka_t,
                                qa[h][:, 512 * a + c_hi - 128:512 * a + c_hi],
                                start=True, stop=False)
                            nc.tensor.matmul(
                                ps[:, hi - 128:hi], ident_sb[:], mnhi_sb[:],
                                start=False, stop=True)
                        else:
                            nc.tensor.matmul(
                                ps[:, lo:hi], ka_t,
                                qa[h][:, qs:512 * a + c_hi],
                                start=True, stop=True)
                    w_t = wexp.tile([128, 512], F16, name=f"w{u}",
                                    tag=f"w{u}")
                    nc.scalar.activation(
                        w_t[:, 0:cw], ps[:, 0:cw],
                        mybir.ActivationFunctionType.Exp, scale=SCALE)
                    wts.append(w_t)
                if len(pend) >= 2:
                    drain_one(pend)
                if first:
                    # older pairs' normalizes hide under this pair's work
                    flush_norms(keep=1)
                    first = False
                pend.append((ctaus, wts))
            while pend:
                drain_one(pend)
            norm_pending.append((a, g, hp, pvs))

        def emit_attn_strip(a):
            chunks = _strip_chunks(a, nt, wt)
            for g in range(GLOC):
                for hp in range(2):
                    emit_attn_pair(a, g, hp, chunks)

        def emit_oproj_strip(a):
            for st in range(4 * a, 4 * a + 4):
                for dcb in range(d // 512):
                    ps = psX.tile([128, 512], F32, name="ps_o", tag="mm")
                    for ec in range(EQ // 128):
                        nc.tensor.matmul(
                            ps[:], oT[ec][:, st * 128:(st + 1) * 128],
                            wo_sb[:, ec, dcb * 512:(dcb + 1) * 512],
                            start=(ec == 0), stop=(ec == EQ // 128 - 1))
                    osb = osbp.tile([128, 512], F16, name="osb", tag="osb")
                    nc.vector.tensor_copy(osb[:], ps[:])
                    nc.sync.dma_start(
                        out_d[st * 128:(st + 1) * 128,
                              dcb * 512:(dcb + 1) * 512], osb[:])

        # ---------- schedule ----------
        # attention strip a only needs proj chunks <= a, so interleave:
        # the Exp engine gets work while the PE runs the next projection.
        for a in range(nstrip):
            emit_proj_chunk(a)
            emit_attn_strip(a)
            if a > 0:
                emit_oproj_strip(a - 1)
        flush_norms()
        emit_oproj_strip(nstrip - 1)

    nc.compile()
    return nc


# ---------------- host-side sharding ----------------

def _prep_core_inputs(c, x, Wq, bq, Wk, bk, Wv, bv, Wo, slopes, s=S, d=D):
    """Build the per-core input map (all numpy, fp16 where declared)."""
    b = c // TP
    hs = c % TP
    f16 = np.float16
    qrows = slice(hs * EQ, (hs + 1) * EQ)
    krows = slice(hs * EKV, (hs + 1) * EKV)
    m = {}
    m["xT"] = np.ascontiguousarray(x[b].T).astype(f16)
    m["wq"] = np.ascontiguousarray(Wq[qrows, :].T).astype(f16)
    m["wk"] = np.ascontiguousarray(Wk[krows, :].T).astype(f16)
    m["wv"] = np.ascontiguousarray(Wv[krows, :].T).astype(f16)
    m["wo"] = np.ascontiguousarray(Wo[:, qrows].T).astype(f16)
    qaug = np.zeros((2 * HLOC, s), np.float32)
    i_idx = np.arange(s, dtype=np.float32)
    for h in range(HLOC):
        sl = float(slopes[hs * HLOC + h])
        qaug[2 * h, :] = sl / SCALE
        qaug[2 * h + 1, :] = -sl / SCALE * i_idx
    m["qaug"] = qaug.astype(f16)
    kaug = np.zeros((2, s), np.float32)
    kaug[0, :] = i_idx
    kaug[1, :] = 1.0
    m["kaug"] = kaug.astype(f16)
    # per-partition bias columns: cols 0..3 q et-blocks, col 4 k
    bcol = np.zeros((128, 5), np.float32)
    for et in range(4):
        bcol[:, et] = bq[qrows][et * 128:(et + 1) * 128]
    bcol[:, 4] = bk[krows]
    m["biascol"] = bcol
    m["vbias"] = bv[krows].astype(f16).reshape(1, -1)
    m["ident"] = np.eye(128, dtype=f16)
    p = np.arange(128)[:, None]
    f = np.arange(128)[None, :]
    m["mnlo"] = np.where(p <= f, 0.0, MNEG).astype(f16)
    m["mnhi"] = np.where(p > f, 0.0, MNEG).astype(f16)
    return m


_PROG_CACHE = {}


def _get_program():
    key = (S, D, WIN)
    if key not in _PROG_CACHE:
        _PROG_CACHE[key] = build_program()
    return _PROG_CACHE[key]


def kernel(hidden_states, Wq, bq, Wk, bk, Wv, bv, Wo, bo, alibi_slopes,
           _want_profile=False):
    x = np.asarray(hidden_states, np.float32)
    Wq = np.asarray(Wq, np.float32)
    Wk = np.asarray(Wk, np.float32)
    Wv = np.asarray(Wv, np.float32)
    Wo = np.asarray(Wo, np.float32)
    bq = np.asarray(bq, np.float32)
    bk = np.asarray(bk, np.float32)
    bv = np.asarray(bv, np.float32)
    bo = np.asarray(bo, np.float32)
    slopes = np.asarray(alibi_slopes, np.float32)

    nc = _get_program()
    in_maps = [
        _prep_core_inputs(c, x, Wq, bq, Wk, bk, Wv, bv, Wo, slopes)
        for c in range(N_CORES)
    ]
    res = run_bass_kernel_spmd(nc, in_maps, list(range(N_CORES)),
                               trace=_want_profile)
    out = np.zeros((B, S, D), np.float32)
    for c in range(N_CORES):
        out[c // TP] += res.results[c]["out"].astype(np.float32)
    out += bo[None, None, :]
    if _want_profile:
        return out, res
    return out
